# revision 13
# baseline (speedup 1.0000x reference)
"""v2: fp8 DoubleRow coarse scoring + strided-packed fold pipeline.

Dispatch A (keys sharded 12800/core, all 2048 queries):
  - prep: exact fp32 key norms (ACT Square + DVE reduce-add), kinv -> DRAM;
    normalize+quantize to fp8 via diag matmul (PE, scale 32*kinv) -> kT fp8.
    x likewise -> xT fp8 (scale 32*xinv).
  - main, per 128-query tile: fp8 DoubleRow matmuls -> PSUM [128,2048] f32
    (= 1024*cosine). Drain into m1p [128, 3200] i32 whose low 16 bits are a
    pre-filled iota (slot id) and high 16 bits receive the f16-rounded score:
      * DVE pa's: tensor_reduce max over PSUM pairs (u, u+1024), f16 out
        written stride-2 into m1p's high bytes.
      * ACT pa's: two ACT copies (PSUM halves -> f16 stride-2 into m1p slot
        and an iota-prefilled scratch), then Pool tt-max on packed i32.
    Packed values compare correctly as f32 (monotone bit trick); iota makes
    every value distinct -> tie-free top-8 with identity, no pack pass.
  - fold2 via gpsimd DMA (copy + accum-max) -> f2 [128, 1600], fold3 in-place
    -> [128, 800], DVE max8 -> top-8 packed candidates [B, 8].

Slot decode (slot = low 16 bits): slot i < 3072: u = i & 511,
row = 4*(i-u) + u + 512*m (m in 0..3);  i >= 3072: row = 12288 + (i-3072)
+ 128*m.  (fold1 pairs (u, u+1024); fold2 pairs (u, u+512) within a pa ->
4 members/slot... NO: see _geom -- members = 2, fold2 crosses pa's.)

Dispatch B (queries sharded 256/core): top-12 of 64 candidates by packed
value (max8/match_replace/max8 + max_index for core id), expand members,
one batched indirect gather of key rows (+inv norm col), exact fp32
rescore, top-8, softmax, batched value gather, weighted sum.
"""

import os
import sys
import time

_TRN_REPO = "/opt/trn_rl_repo"
if _TRN_REPO not in sys.path:
    sys.path.insert(0, _TRN_REPO)

import numpy as np

import concourse.bass as bass
import concourse.mybir as mybir
import concourse.tile as tile
from concourse import bacc
from concourse.bass import IndirectOffsetOnAxis
from concourse.bass_utils import run_bass_kernel_spmd
from concourse.masks import make_identity

F32 = mybir.dt.float32
F16 = mybir.dt.float16
I16 = mybir.dt.int16
FP8 = mybir.dt.float8e4
I32 = mybir.dt.int32
U32 = mybir.dt.uint32
ALU = mybir.AluOpType
ACTF = mybir.ActivationFunctionType
AX = mybir.AxisListType

B = 2048
D = 256
N = 100000
TOPK = 8
NCORES = 8
NLOC = 12800              # 25 ktiles of 512... (25*512=12800); 8*12800 >= N
NPAD = NLOC * NCORES
BSLOTS = 24
NMEMB = BSLOTS * 2

_NEG_BIG = -3.0e38
QSCALE = 32.0
V_TTR = os.environ.get("V_TTR", "0") == "1"  # ttr ucode op fails on HW
V_DR = os.environ.get("V_DR", "1") == "1"          # DoubleRow matmul
V_STRIDED = os.environ.get("V_STRIDED", "1") == "1"  # strided f16 hi-byte writes


def _geom(nloc):
    # pa blocks of 1024 keys + one leftover block of 512
    npa = nloc // 1024            # full pa's
    rem = nloc - npa * 1024       # 512 leftover
    assert rem in (0, 512)
    m1w = npa * 512 + rem // 2    # after fold1 (2 members per slot)
    return npa, rem, m1w


# m1 slot i (members 2):
#   i < npa*512: g = i >> 9, u = i & 511 -> rows g*1024 + u + 512*m
#   i >= npa*512: u = i - npa*512 -> rows npa*1024 + u + 256*m


# --------------------------------------------------------------------------
# Dispatch A
# --------------------------------------------------------------------------

def build_dispatch_a(bq=B, nloc=NLOC, dbg=False):
    npa, rem, m1w = _geom(nloc)
    qtiles = bq // 128
    ktiles = nloc // 128
    f2w = m1w // 2
    f3w = f2w // 2
    assert m1w % 4 == 0 and m1w < 65536

    # drain split: DVE reduce-pair pa's vs ACT-copy pa's (+ Pool fold)
    n_dve_pa = 0 if npa >= 8 else max(npa - 1, 0)

    def _hi16(tile_i32, lo, hi):
        # f16 view of the high bytes of i32 words [lo, hi)
        v = tile_i32[:].bitcast(I16).rearrange("p (n two) -> p n two", two=2)
        return v[:, lo:hi, 1].bitcast(F16)

    nc = bacc.Bacc("TRN2", target_bir_lowering=False, debug=dbg)
    x_d = nc.dram_tensor("x", [bq, D], F32, kind="ExternalInput").ap()
    k_d = nc.dram_tensor("keys", [nloc, D], F32, kind="ExternalInput").ap()
    out_d = nc.dram_tensor("cand", [bq, 8], F32, kind="ExternalOutput").ap()
    kinv_d = nc.dram_tensor("kinv", [nloc, 1], F32, kind="ExternalOutput").ap()

    with tile.TileContext(nc) as tc:
        with (
            tc.tile_pool(name="const", bufs=1) as constp,
            tc.tile_pool(name="kprep", bufs=4) as kprep,
            tc.tile_pool(name="big", bufs=1) as bigp,
            tc.tile_pool(name="xp", bufs=2) as xp,
            tc.tile_pool(name="m1", bufs=2) as m1pool,
            tc.tile_pool(name="f2", bufs=2) as f2pool,
            tc.tile_pool(name="sc", bufs=2) as scpool,
            tc.tile_pool(name="ps", bufs=3, space="PSUM") as psp,
            tc.tile_pool(name="pst", bufs=2, space="PSUM") as pst,
        ):
            identf = constp.tile([128, 128], F32)
            make_identity(nc, identf[:])
            identf8 = constp.tile([128, 128], FP8)
            make_identity(nc, identf8[:])
            eps = constp.tile([128, 1], F32)
            nc.gpsimd.memset(eps[:], 1e-30)

            kT = bigp.tile([128, 2, nloc], FP8)
            xT = bigp.tile([128, 2, bq], FP8)
            kinv_all = bigp.tile([128, ktiles], F32)

            # iota-prefilled packed buffers (low 16 bits = slot id).
            # buffer 0 + scratch early (qtile 0 needs them); buffer 1 after
            # key prep so Pool's diag mults aren't delayed.
            m1ps = [bigp.tile([128, m1w], I32, tag=f"m1p{bi}", name=f"m1p{bi}")
                    for bi in range(2)]
            nc.gpsimd.iota(m1ps[0][:], pattern=[[1, m1w]], base=0,
                           channel_multiplier=0)
            n_act_pa = npa - n_dve_pa
            scr_w = n_act_pa * 512
            scr0 = bigp.tile([128, max(scr_w, 4)], I32, tag="scr0")
            if scr_w:
                nc.gpsimd.iota(scr0[:, :scr_w],
                               pattern=[[1, scr_w]],
                               base=n_dve_pa * 512, channel_multiplier=0)
            scrs = [scr0, scr0]

            # ---- x prep ----
            for qt in range(qtiles):
                xt = xp.tile([128, D], F32, tag="xt")
                nc.sync.dma_start(out=xt[:], in_=x_d[qt * 128:(qt + 1) * 128, :])
                xsq = xp.tile([128, D], F32, tag="xsq")
                xn2 = xp.tile([128, 1], F32, tag="xn2")
                if V_TTR:
                    nc.vector.tensor_tensor_reduce(
                        out=xsq[:], in0=xt[:], in1=xt[:], scale=1.0,
                        scalar=0.0, op0=ALU.mult, op1=ALU.add,
                        accum_out=xn2[:])
                else:
                    nc.scalar.activation(xsq[:], xt[:], ACTF.Square,
                                         accum_out=xn2[:])
                xsrt = xp.tile([128, 1], F32, tag="xsrt")
                nc.scalar.activation(xsrt[:], xn2[:], ACTF.Sqrt)
                xinv = xp.tile([128, 1], F32, tag="xinv")
                nc.vector.reciprocal(xinv[:], xsrt[:])
                xinv32 = xp.tile([128, 1], F32, tag="xinv32")
                nc.vector.tensor_scalar(xinv32[:], xinv[:], QSCALE, None,
                                        op0=ALU.mult)
                xq = xp.tile([128, D], FP8, tag="xq")
                nc.scalar.activation(xq[:], xt[:], ACTF.Copy, scale=xinv32[:])
                pt = pst.tile([128, 512], F32, tag="pt")
                for c in range(2):
                    nc.tensor.matmul(pt[:, c * 128:(c + 1) * 128],
                                     lhsT=xq[:, c * 128:(c + 1) * 128],
                                     rhs=identf8[:], start=True, stop=True)
                for c in range(2):
                    nc.vector.tensor_copy(xT[:, c, qt * 128:(qt + 1) * 128],
                                          pt[:, c * 128:(c + 1) * 128])

            # ---- key prep: batches of 4 ktiles ----
            nb = ktiles // 4
            for g in range(nb):
                ktf = kprep.tile([128, 4, D], F32, tag="ktf")
                nc.sync.dma_start(
                    out=ktf[:],
                    in_=k_d[g * 512:(g + 1) * 512, :].rearrange(
                        "(f p) d -> p f d", p=128))
                ksq = kprep.tile([128, 4, D], F32, tag="ksq")
                nc.scalar.activation(ksq[:], ktf[:], ACTF.Square)
                kn2 = kprep.tile([128, 4], F32, tag="kn2")
                nc.vector.tensor_reduce(kn2[:], ksq[:], axis=AX.X, op=ALU.add)
                ksrt = kprep.tile([128, 4], F32, tag="ksrt")
                nc.scalar.activation(ksrt[:], kn2[:], ACTF.Sqrt, bias=eps[:])
                kinvg = kprep.tile([128, 4], F32, tag="kinvg")
                nc.vector.reciprocal(kinvg[:], ksrt[:])
                nc.vector.tensor_copy(kinv_all[:, g * 4:(g + 1) * 4], kinvg[:])
                kinv32 = kprep.tile([128, 4], F32, tag="kinv32")
                nc.vector.tensor_scalar(kinv32[:], kinvg[:], QSCALE, None,
                                        op0=ALU.mult)
                for h in range(2):
                    pt = pst.tile([128, 512], F32, tag="pt")
                    for i2 in range(2):
                        i = h * 2 + i2
                        diag = kprep.tile([128, 128], F32, tag=f"diag{i}")
                        nc.gpsimd.tensor_tensor(
                            diag[:], identf[:],
                            kinv32[:, i:i + 1].to_broadcast([128, 128]),
                            op=ALU.mult)
                        for c in range(2):
                            nc.tensor.matmul(
                                pt[:, i2 * 256 + c * 128:
                                   i2 * 256 + (c + 1) * 128],
                                lhsT=ktf[:, i, c * 128:(c + 1) * 128],
                                rhs=diag[:], start=True, stop=True)
                    # pt layout [i2, c, 128]
                    ptv = pt[:].rearrange("p (i c q) -> p i c q", c=2, q=128)
                    for c in range(2):
                        dst = kT[:, c, g * 512 + h * 256:
                                 g * 512 + (h + 1) * 256].rearrange(
                            "p (i q) -> p i q", q=128)
                        if g % 2 == 0:
                            nc.scalar.activation(dst, ptv[:, :, c, :],
                                                 ACTF.Copy)
                        else:
                            nc.vector.tensor_copy(dst, ptv[:, :, c, :])
            nc.sync.dma_start(
                out=kinv_d[:].rearrange("(t p) o -> p (t o)", p=128),
                in_=kinv_all[:])

            nc.gpsimd.iota(m1ps[1][:], pattern=[[1, m1w]], base=0,
                           channel_multiplier=0)


            # ---- main loop ----
            for qt in range(qtiles):
                m1p = m1ps[qt % 2]
                scr = scrs[qt % 2]
                lhsT = xT[:, :, qt * 128:(qt + 1) * 128]

                for g in range(npa):
                    pa = psp.tile([128, 1024], F32, tag="pa")
                    for i in range(2):
                        rhs = kT[:, :, g * 1024 + i * 512:
                                 g * 1024 + (i + 1) * 512]
                        if V_DR:
                            nc.tensor.matmul(
                                pa[:, i * 512:(i + 1) * 512], lhsT=lhsT,
                                rhs=rhs, start=True, stop=True,
                                perf_mode=mybir.MatmulPerfMode.DoubleRow)
                        else:
                            for c in range(2):
                                nc.tensor.matmul(
                                    pa[:, i * 512:(i + 1) * 512],
                                    lhsT=lhsT[:, c], rhs=rhs[:, c],
                                    start=(c == 0), stop=(c == 1))
                    if g < n_dve_pa:
                        dst = _hi16(m1p, g * 512, (g + 1) * 512)
                        nc.vector.tensor_reduce(
                            dst,
                            pa[:].rearrange("p (two n) -> p n two", two=2),
                            axis=AX.X, op=ALU.max)
                    else:
                        ga = g - n_dve_pa
                        d0 = _hi16(m1p, g * 512, (g + 1) * 512)
                        d1 = _hi16(scr, ga * 512, (ga + 1) * 512)
                        nc.scalar.activation(d0, pa[:, :512], ACTF.Copy)
                        nc.scalar.activation(d1, pa[:, 512:], ACTF.Copy)
                        nc.vector.tensor_tensor(
                            m1p[:, g * 512:(g + 1) * 512].bitcast(F32),
                            m1p[:, g * 512:(g + 1) * 512].bitcast(F32),
                            scr[:, ga * 512:(ga + 1) * 512].bitcast(F32),
                            op=ALU.max)
                if rem:
                    pr = pst.tile([128, 512], F32, tag="pt")
                    rhsr = kT[:, :, npa * 1024:npa * 1024 + 512]
                    if V_DR:
                        nc.tensor.matmul(
                            pr[:], lhsT=lhsT, rhs=rhsr,
                            start=True, stop=True,
                            perf_mode=mybir.MatmulPerfMode.DoubleRow)
                    else:
                        for c in range(2):
                            nc.tensor.matmul(
                                pr[:], lhsT=lhsT[:, c], rhs=rhsr[:, c],
                                start=(c == 0), stop=(c == 1))
                    dst = _hi16(m1p, npa * 512, m1w)
                    nc.vector.tensor_reduce(
                        dst, pr[:].rearrange("p (two n) -> p n two", two=2),
                        axis=AX.X, op=ALU.max)

                # fold2 (split DVE/Pool) then fold3 + top8 on DVE
                f2 = f2pool.tile([128, f2w], F32, tag="f2")
                nc.vector.tensor_tensor(
                    f2[:], m1p[:, :f2w].bitcast(F32),
                    m1p[:, f2w:].bitcast(F32), op=ALU.max)
                nc.vector.tensor_tensor(
                    f2[:, :f3w], f2[:, :f3w], f2[:, f3w:], op=ALU.max)
                top = scpool.tile([128, 8], F32, tag="top")
                nc.vector.max(out=top[:], in_=f2[:, :f3w])
                nc.sync.dma_start(out=out_d[qt * 128:(qt + 1) * 128, :],
                                  in_=top[:])

    nc.compile()
    return nc


# --------------------------------------------------------------------------
# Dispatch B
# --------------------------------------------------------------------------

def build_dispatch_b(bq_slice, nloc=NLOC, npad=NPAD, ncand=NCORES * 8,
                     bslots=BSLOTS, dbg=False):
    qtiles = bq_slice // 128
    npa, rem, m1w = _geom(nloc)
    nmemb = bslots * 2
    DA = D + 1
    lim = float(npa * 512)    # slots below lim: stride 512; above: 256

    nc = bacc.Bacc("TRN2", target_bir_lowering=False, debug=dbg)
    v_d = nc.dram_tensor("vals", [bq_slice, ncand], F32, kind="ExternalInput").ap()
    x_d = nc.dram_tensor("x", [bq_slice, D], F32, kind="ExternalInput").ap()
    k_d = nc.dram_tensor("keysaug", [npad, DA], F32, kind="ExternalInput").ap()
    val_d = nc.dram_tensor("values", [npad, D], F32, kind="ExternalInput").ap()
    out_d = nc.dram_tensor("out", [bq_slice, D], F32, kind="ExternalOutput").ap()

    with tile.TileContext(nc) as tc:
        with (
            tc.tile_pool(name="const", bufs=1) as constp,
            tc.tile_pool(name="wp", bufs=2) as wp,
            tc.tile_pool(name="gp", bufs=2) as gp,
        ):
            iota_m_i = constp.tile([128, nmemb], I32)
            nc.gpsimd.iota(iota_m_i[:], pattern=[[1, nmemb]], base=0,
                           channel_multiplier=0)
            iota_m_f = constp.tile([128, nmemb], F32)
            nc.gpsimd.tensor_copy(iota_m_f[:], iota_m_i[:])

            for qt in range(qtiles):
                r0, r1 = qt * 128, (qt + 1) * 128

                # --- x_norm exact fp32 ---
                xt = wp.tile([128, D], F32, tag="xt")
                nc.sync.dma_start(out=xt[:], in_=x_d[r0:r1, :])
                xsq = wp.tile([128, D], F32, tag="xsq")
                xn2 = wp.tile([128, 1], F32, tag="xn2")
                nc.scalar.activation(xsq[:], xt[:], ACTF.Square, accum_out=xn2[:])
                xsrt = wp.tile([128, 1], F32, tag="xsrt")
                nc.scalar.activation(xsrt[:], xn2[:], ACTF.Sqrt)
                xinv = wp.tile([128, 1], F32, tag="xinv")
                nc.vector.reciprocal(xinv[:], xsrt[:])
                xn = wp.tile([128, D], F32, tag="xn")
                nc.scalar.activation(xn[:], xt[:], ACTF.Copy, scale=xinv[:])

                # --- top-(bslots) of the 64 packed candidates ---
                vin = wp.tile([128, ncand], F32, tag="vin")
                nc.sync.dma_start(out=vin[:], in_=v_d[r0:r1, :])
                t16 = wp.tile([128, 24], F32, tag="t16")
                nc.vector.max(out=t16[:, 0:8], in_=vin[:])
                vrep = wp.tile([128, ncand], F32, tag="vrep")
                nc.vector.match_replace(out=vrep[:], in_to_replace=t16[:, 0:8],
                                        in_values=vin[:], imm_value=_NEG_BIG)
                nc.vector.max(out=t16[:, 8:16], in_=vrep[:])
                vrep2 = wp.tile([128, ncand], F32, tag="vrep2")
                nc.vector.match_replace(out=vrep2[:],
                                        in_to_replace=t16[:, 8:16],
                                        in_values=vrep[:], imm_value=_NEG_BIG)
                nc.vector.max(out=t16[:, 16:24], in_=vrep2[:])
                pos16 = wp.tile([128, 24], U32, tag="pos16")
                nc.vector.max_index(pos16[:, 0:8], t16[:, 0:8], vin[:])
                nc.vector.max_index(pos16[:, 8:16], t16[:, 8:16], vrep[:])
                nc.vector.max_index(pos16[:, 16:24], t16[:, 16:24], vrep2[:])

                # --- decode: slot, core base, member rows ---
                j_i = wp.tile([128, bslots], I32, tag="ji")
                nc.vector.tensor_scalar(j_i[:], t16[:, :bslots].bitcast(I32),
                                        0xFFFF, None, op0=ALU.bitwise_and)
                j_f = wp.tile([128, bslots], F32, tag="jf")
                nc.vector.tensor_copy(j_f[:], j_i[:])
                pos_i = wp.tile([128, bslots], I32, tag="posi")
                nc.vector.tensor_scalar(pos_i[:], pos16[:, :bslots].bitcast(I32),
                                        3, None, op0=ALU.logical_shift_right)
                pos_f = wp.tile([128, bslots], F32, tag="posf")
                nc.vector.tensor_copy(pos_f[:], pos_i[:])
                cb_f = wp.tile([128, bslots], F32, tag="cbf")
                nc.vector.tensor_scalar(cb_f[:], pos_f[:], float(nloc), None,
                                        op0=ALU.mult)

                # u = j & 511 (full pa region); rows0 = cb + 2*(j-u) + u
                u_i = wp.tile([128, bslots], I32, tag="ui")
                nc.vector.tensor_scalar(u_i[:], j_i[:], 511, None,
                                        op0=ALU.bitwise_and)
                u_f = wp.tile([128, bslots], F32, tag="uf")
                nc.vector.tensor_copy(u_f[:], u_i[:])
                rows_full = wp.tile([128, bslots], F32, tag="rowsfull")
                nc.vector.tensor_tensor(rows_full[:], j_f[:], u_f[:],
                                        op=ALU.subtract)
                nc.vector.tensor_scalar(rows_full[:], rows_full[:], 2.0, None,
                                        op0=ALU.mult)
                nc.vector.tensor_tensor(rows_full[:], rows_full[:], u_f[:],
                                        op=ALU.add)
                # leftover region: rows = npa*1024 + (j - lim)
                rows_rem = wp.tile([128, bslots], F32, tag="rowsrem")
                nc.vector.tensor_scalar(rows_rem[:], j_f[:],
                                        float(npa * 1024) - lim, None,
                                        op0=ALU.add)
                isfull = wp.tile([128, bslots], F32, tag="isfull")
                nc.vector.tensor_scalar(isfull[:], j_f[:], lim, None,
                                        op0=ALU.is_lt)
                rows0 = wp.tile([128, bslots], F32, tag="rows0")
                # rows0 = isfull ? rows_full : rows_rem
                nc.vector.tensor_tensor(rows_full[:], rows_full[:], rows_rem[:],
                                        op=ALU.subtract)
                nc.vector.tensor_tensor(rows0[:], isfull[:], rows_full[:],
                                        op=ALU.mult)
                nc.vector.tensor_tensor(rows0[:], rows0[:], rows_rem[:],
                                        op=ALU.add)
                nc.vector.tensor_tensor(rows0[:], rows0[:], cb_f[:], op=ALU.add)
                # member stride: isfull ? 512 : 256
                stride = wp.tile([128, bslots], F32, tag="stride")
                nc.vector.tensor_scalar(stride[:], isfull[:], 256.0, 256.0,
                                        op0=ALU.mult, op1=ALU.add)

                rows_f = wp.tile([128, nmemb], F32, tag="rowsf")
                nc.vector.tensor_copy(rows_f[:, :bslots], rows0[:])
                nc.vector.tensor_tensor(rows_f[:, bslots:], rows0[:], stride[:],
                                        op=ALU.add)
                rows_i = wp.tile([128, nmemb], I32, tag="rowsi")
                nc.vector.tensor_copy(rows_i[:], rows_f[:])

                # --- batched gather + exact rescore ---
                g = gp.tile([128, nmemb, DA], F32, tag="g")
                for sidx in range(nmemb):
                    nc.gpsimd.indirect_dma_start(
                        out=g[:, sidx, :], out_offset=None, in_=k_d[:],
                        in_offset=IndirectOffsetOnAxis(
                            ap=rows_i[:, sidx:sidx + 1], axis=0))
                xb = xn[:].unsqueeze(1)
                nc.vector.tensor_tensor(g[:, :, :D], g[:, :, :D],
                                        xb.to_broadcast([128, nmemb, D]),
                                        op=ALU.mult)
                dotm = wp.tile([128, nmemb], F32, tag="dotm")
                nc.vector.tensor_reduce(dotm[:], g[:, :, :D], axis=AX.X,
                                        op=ALU.add)
                sco = wp.tile([128, nmemb], F32, tag="sco")
                nc.vector.tensor_tensor(sco[:], dotm[:], g[:, :, D], op=ALU.mult)

                # --- exact top-8 ---
                top8 = wp.tile([128, 8], F32, tag="top8")
                nc.vector.max(out=top8[:], in_=sco[:])
                pos8 = wp.tile([128, 8], U32, tag="pos8")
                nc.vector.max_index(pos8[:], top8[:], sco[:])
                pos8f = wp.tile([128, 8], F32, tag="pos8f")
                nc.vector.tensor_copy(pos8f[:], pos8[:])

                # --- softmax over raw cosine scores ---
                sh = wp.tile([128, 8], F32, tag="sh")
                nc.vector.tensor_tensor(sh[:], top8[:],
                                        top8[:, 0:1].to_broadcast([128, 8]),
                                        op=ALU.subtract)
                ex = wp.tile([128, 8], F32, tag="ex")
                nc.scalar.activation(ex[:], sh[:], ACTF.Exp)
                es = wp.tile([128, 1], F32, tag="es")
                nc.vector.tensor_reduce(es[:], ex[:], axis=AX.X, op=ALU.add)
                esr = wp.tile([128, 1], F32, tag="esr")
                nc.vector.reciprocal(esr[:], es[:])
                wgt = wp.tile([128, 8], F32, tag="wgt")
                nc.vector.tensor_tensor(wgt[:], ex[:],
                                        esr[:].to_broadcast([128, 8]),
                                        op=ALU.mult)

                # --- winner rows via one-hot over member index ---
                winr = wp.tile([128, 8], F32, tag="winr")
                ohm = wp.tile([128, nmemb], F32, tag="ohm")
                for w in range(8):
                    nc.vector.tensor_tensor(
                        ohm[:], iota_m_f[:],
                        pos8f[:, w:w + 1].to_broadcast([128, nmemb]),
                        op=ALU.is_equal)
                    nc.vector.tensor_tensor(ohm[:], ohm[:], rows_f[:],
                                            op=ALU.mult)
                    nc.vector.tensor_reduce(winr[:, w:w + 1], ohm[:], axis=AX.X,
                                            op=ALU.add)
                winr_i = wp.tile([128, 8], I32, tag="winri")
                nc.vector.tensor_copy(winr_i[:], winr[:])

                # --- batched value gather, weighted sum ---
                vg = gp.tile([128, 8, D], F32, tag="vg")
                for k8 in range(8):
                    nc.gpsimd.indirect_dma_start(
                        out=vg[:, k8, :], out_offset=None, in_=val_d[:],
                        in_offset=IndirectOffsetOnAxis(
                            ap=winr_i[:, k8:k8 + 1], axis=0))
                vw = gp.tile([128, 8, D], F32, tag="vw")
                nc.vector.tensor_tensor(
                    vw[:], vg[:],
                    wgt[:].unsqueeze(2).to_broadcast([128, 8, D]), op=ALU.mult)
                ot = wp.tile([128, D], F32, tag="ot")
                nc.vector.tensor_reduce(ot[:], vw[:].rearrange("p k d -> p d k"),
                                        axis=AX.X, op=ALU.add)
                nc.sync.dma_start(out=out_d[r0:r1, :], in_=ot[:])

    nc.compile()
    return nc


# --------------------------------------------------------------------------
# Host orchestration
# --------------------------------------------------------------------------

_CACHE = {}
TRACE = False
last_exec_ns = (None, None)


def _run(nc, in_maps, core_ids):
    if TRACE:
        return run_bass_kernel_spmd(nc, in_maps, core_ids, trace=True)
    return run_bass_kernel_spmd(nc, in_maps, core_ids)


def _get_programs():
    if "A" not in _CACHE:
        _CACHE["A"] = build_dispatch_a()
    if "B" not in _CACHE:
        _CACHE["B"] = build_dispatch_b(B // NCORES)
    return _CACHE["A"], _CACHE["B"]


def kernel(x, keys, values, top_k):
    assert int(top_k) == TOPK
    x = np.ascontiguousarray(np.asarray(x, dtype=np.float32))
    keys = np.asarray(keys, dtype=np.float32)
    values = np.asarray(values, dtype=np.float32)
    assert x.shape == (B, D) and keys.shape == (N, D) and values.shape == (N, D)

    keys_pad = np.zeros((NPAD, D), dtype=np.float32)
    keys_pad[:N] = keys
    values_pad = np.zeros((NPAD, D), dtype=np.float32)
    values_pad[:N] = values

    nc_a, nc_b = _get_programs()
    core_ids = list(range(NCORES))

    in_maps_a = [
        {"x": x, "keys": np.ascontiguousarray(keys_pad[c * NLOC:(c + 1) * NLOC])}
        for c in range(NCORES)
    ]
    t0 = time.perf_counter()
    res_a = _run(nc_a, in_maps_a, core_ids)
    t1 = time.perf_counter()
    cand = np.concatenate([res_a.results[c]["cand"] for c in range(NCORES)],
                          axis=1)
    kinv = np.concatenate([res_a.results[c]["kinv"] for c in range(NCORES)],
                          axis=0)
    keys_aug = np.ascontiguousarray(
        np.concatenate([keys_pad, kinv.reshape(NPAD, 1)], axis=1))

    bs = B // NCORES
    in_maps_b = [
        {
            "vals": np.ascontiguousarray(cand[c * bs:(c + 1) * bs]),
            "x": np.ascontiguousarray(x[c * bs:(c + 1) * bs]),
            "keysaug": keys_aug,
            "values": values_pad,
        }
        for c in range(NCORES)
    ]
    t2 = time.perf_counter()
    res_b = _run(nc_b, in_maps_b, core_ids)
    t3 = time.perf_counter()
    out = np.concatenate([res_b.results[c]["out"] for c in range(NCORES)],
                         axis=0)
    kernel.last_walltimes = (t1 - t0, t3 - t2)
    if TRACE:
        global last_exec_ns
        last_exec_ns = (res_a.exec_time_ns, res_b.exec_time_ns)
    return out.astype(np.float32)
